# revision 13
# baseline (speedup 1.0000x reference)
"""Trainium2 Bass kernel for ExtractRelevantPatches (pool -> top-k -> gather).

Full-input contract: kernel(heatmap [64,448,448,1] f32, image [64,448,448,3] f32)
-> [1344, 64, 64, 3] f32.

Sharding: pure data-parallel over batch; 8 batches per NeuronCore, 8 cores.

Per-core algorithm (raw Bass, explicit semaphores), v4 — pipelined over 4
groups of 2 batches, whole top-k/index chain on-chip:

  Per group g (batches 2g, 2g+1; b' = par):
  1. Heatmap -> SBUF [128, 7, 448], partition p = 64*par + r. The two
     partition halves ride DIFFERENT HWDGE rings (par0 on SP/sync, par1 on
     ACT/scalar) so even- and odd-port halves of the SBUF fabric stream
     concurrently (one 64-partition DMA reaches only half the AXI ports).
  2. One DVE reduce_sum over 64-col groups -> red [128, 49].
  3. One matmul with selector F2 [128, 2] (F2[p,i] = 1 iff i == p//64)
     -> PSUM psV [2, 49] = per-batch pooled sums. No DRAM shuffle.
  4. Top-24 via 3x (max + max_index + match_replace); keep first 21.
  5. base[b',k] = idx + 441*(idx//7) (patch-row units), cast to f32.
  6. 2 broadcast matmuls (lhsT = one-hot row selector E2 [2, 128]) spread
     base to all 128 partitions -> PSUM psD [128, 42]; one DVE
     tensor_tensor add (x4 stride-0 broadcast) with f32 static table
     (7*(p%16) + 112*q + 3136*b) -> int16 idx16 slice [128, 168].
  7. dma_gather chunks on 4 SWDGE queues, graduated sizes
     (128,256,512,896,896) per group for a fast ramp (desc-gen runs
     ~8.5ns/idx on one Q7 core per chunk).
  8. One store per gather chunk on the SP ring (free after par0 loads),
     AP out[(c p) e -> p c e]. Per-queue FIFO completion -> 4 counting
     sems cover all 20 chunks.

  All constants ship as ONE packed SWDGE DMA on the gpsimd path so
  neither HWDGE ring pays the small-DMA fixed-latency at its head.
"""

import numpy as np

_N_CORES = 8
_B = 64
_B_LOC = _B // _N_CORES  # 8
_PATCH = 64
_GRID = 7
_NPATCH = 21
_PROW = _PATCH * 3            # 192 elements per patch-row
_OUT_ROWS_LOC = _B_LOC * _NPATCH  # 168
_NGRP = 4                     # batch groups per core
_BG = _B_LOC // _NGRP         # 2 batches per group

# gather chunk sizes per group, in indices (multiples of 128; sum = 2688)
_CHUNKS = [128, 256, 512, 896, 896]
_NCH_G = len(_CHUNKS)
_NCHUNK = _NGRP * _NCH_G      # 20

_nc_cache = None


def build_program():
    """Build the per-core SPMD Bass program (cached)."""
    global _nc_cache
    if _nc_cache is not None:
        return _nc_cache

    import concourse.bass as bass
    import concourse.bacc as bacc
    import concourse.mybir as mybir

    f32 = mybir.dt.float32
    i16 = mybir.dt.int16
    i32 = mybir.dt.int32
    u32 = mybir.dt.uint32
    X = mybir.AxisListType.X
    Op = mybir.AluOpType

    nc = bacc.Bacc(num_swdge_queues=4)

    hm_in = nc.declare_dram_parameter(
        "heatmap", [_B_LOC, 448, 448, 1], f32, isOutput=False)
    img_in = nc.declare_dram_parameter(
        "image", [_B_LOC, 448, 448, 3], f32, isOutput=False)
    out_t = nc.declare_dram_parameter(
        "out", [_OUT_ROWS_LOC, _PATCH, _PATCH, 3], f32, isOutput=True)

    # --- one packed inline constant [128, 930] f32 ------------------------
    # cols 0:2    F2 selector  (F2[p, i] = 1 iff i == p//64)
    # cols 2:258  E2 selectors (E2_bl[p, i] = 1 iff p == bl), partitions 0:2
    # cols 258:930 static gather-index table: position i = R at
    #   [R%16, R//16]; col s: term = 7*(R%16) + 112*(s%4) + 3136*(s//84)
    pk = np.zeros((128, 930), dtype=np.float32)
    pk[:64, 0] = 1.0
    pk[64:, 1] = 1.0
    pk[0, 2:130] = 1.0
    pk[1, 130:258] = 1.0
    s_ar = np.arange(672, dtype=np.int64)
    w_ar = np.arange(16, dtype=np.int64)
    st = (112 * (s_ar[None, :] % 4) + 7 * w_ar[:, None]
          + 3136 * (s_ar[None, :] // 84)).astype(np.float32)
    pk[:, 258:930] = np.tile(st, (8, 1))
    pk_const = nc.inline_tensor(pk, name="pk_const")

    # --- DRAM views -------------------------------------------------------
    img_rows = (img_in[:]
                .rearrange("b r c ch -> (b r c ch)")
                .rearrange("(n e) -> n e", e=_PROW))

    out_pc = (out_t[:]
              .rearrange("r a b c -> (r a b c)")
              .rearrange("(n e) -> n e", e=_PROW)
              .rearrange("(c p) e -> p c e", p=128))

    # heatmap per-group views: [par, 64, 7, 448]
    hm_src = []
    for g in range(_NGRP):
        hm_src.append(
            hm_in[2 * g:2 * (g + 1)]
            .rearrange("par (br r) c one -> par r br (c one)", r=64))

    # per-chunk geometry: (group, idx16 col offset, idx16 col width,
    #                      GT col offset, GT col width, num idxs)
    chunk_geo = []
    for g in range(_NGRP):
        off16 = 168 * g
        offGT = 21 * g
        for n in _CHUNKS:
            chunk_geo.append((g, off16, n // 16, offGT, n // 128, n))
            off16 += n // 16
            offGT += n // 128

    from contextlib import ExitStack

    with ExitStack() as ctx:
        e = ctx.enter_context
        hm = [e(nc.sbuf_tensor(f"hm{g}", [128, 7, 448], f32))
              for g in range(_NGRP)]
        red = [e(nc.sbuf_tensor(f"red{g}", [128, 49], f32))
               for g in range(_NGRP)]
        pk_sb = e(nc.sbuf_tensor("pk_sb", [128, 930], f32))
        V = [e(nc.sbuf_tensor(f"V{g}", [2, 49], f32)) for g in range(_NGRP)]
        vwork = [e(nc.sbuf_tensor(f"vwork{g}", [2, 49], f32))
                 for g in range(_NGRP)]
        m8 = [e(nc.sbuf_tensor(f"m8_{g}", [2, 8], f32)) for g in range(_NGRP)]
        idx_u = [e(nc.sbuf_tensor(f"idx_u{g}", [2, 24], u32))
                 for g in range(_NGRP)]
        idx_i = [e(nc.sbuf_tensor(f"idx_i{g}", [2, _NPATCH], i32))
                 for g in range(_NGRP)]
        br_i = [e(nc.sbuf_tensor(f"br_i{g}", [2, _NPATCH], i32))
                for g in range(_NGRP)]
        base_bk = [e(nc.sbuf_tensor(f"base_bk{g}", [2, _NPATCH], i32))
                   for g in range(_NGRP)]
        base_f = [e(nc.sbuf_tensor(f"base_f{g}", [2, _NPATCH], f32))
                  for g in range(_NGRP)]
        idx16 = e(nc.sbuf_tensor("idx16", [128, 672], i16))
        GT = e(nc.sbuf_tensor("GT", [128, 84, _PROW], f32))
        psV = [e(nc.psum_tensor(f"psV{g}", [2, 49], f32))
               for g in range(_NGRP)]
        psD = [e(nc.psum_tensor(f"psD{g}", [128, 42], f32))
               for g in range(_NGRP)]

        s_ld = [e(nc.semaphore(f"s_ld{g}")) for g in range(_NGRP)]
        s_red = [e(nc.semaphore(f"s_red{g}")) for g in range(_NGRP)]
        s_mmV = [e(nc.semaphore(f"s_mmV{g}")) for g in range(_NGRP)]
        s_base = [e(nc.semaphore(f"s_base{g}")) for g in range(_NGRP)]
        s_mmD = [e(nc.semaphore(f"s_mmD{g}")) for g in range(_NGRP)]
        s_idx = [e(nc.semaphore(f"s_idx{g}")) for g in range(_NGRP)]
        s_cst = e(nc.semaphore("s_cst"))
        s_gq = [e(nc.semaphore(f"s_gq{q}")) for q in range(4)]
        s_st = e(nc.semaphore("s_st"))
        block = e(nc.Block())

        @block.sync
        def _(sync):
            # par0 heatmap halves on the SP ring, then the stores
            for g in range(_NGRP):
                sync.dma_start(
                    out=hm[g][0:64, :, :],
                    in_=hm_src[g][0:1, :, :, :]
                    .rearrange("one r br c -> (one r) br c"),
                ).then_inc(s_ld[g], 16)
            for c, (g, o16, w16, oGT, wGT, n) in enumerate(chunk_geo):
                sync.wait_ge(s_gq[c % 4], 16 * (c // 4 + 1))
                sync.dma_start(
                    out=out_pc[:, oGT:oGT + wGT, :],
                    in_=GT[:, oGT:oGT + wGT, :],
                ).then_inc(s_st, 16)
            sync.wait_ge(s_st, 16 * _NCHUNK)

        @block.scalar
        def _(sc):
            # par1 heatmap halves on the ACT ring (nothing else ahead)
            for g in range(_NGRP):
                sc.dma_start(
                    out=hm[g][64:128, :, :],
                    in_=hm_src[g][1:2, :, :, :]
                    .rearrange("one r br c -> (one r) br c"),
                ).then_inc(s_ld[g], 16)

        @block.vector
        def _(vector):
            for g in range(_NGRP):
                vector.wait_ge(s_ld[g], 32)
                vector.reduce_sum(
                    out=red[g][:],
                    in_=hm[g][:].rearrange("p br (bc u) -> p (br bc) u", u=64),
                    axis=X,
                )
                vector.drain().then_inc(s_red[g], 1)
                # PSUM -> SBUF pooled sums
                vector.wait_ge(s_mmV[g], 1)
                vector.tensor_copy(out=V[g][:], in_=psV[g][:])
                vector.drain()
                # top-24, keep 21
                cur = V[g]
                for r3 in range(3):
                    vector.max(out=m8[g][:], in_=cur[:])
                    vector.drain()
                    vector.max_index(
                        out=idx_u[g][:, 8 * r3:8 * r3 + 8], in_max=m8[g][:],
                        in_values=cur[:])
                    if r3 < 2:
                        nxt = vwork[g] if r3 == 0 else V[g]
                        vector.match_replace(
                            out=nxt[:], in_to_replace=m8[g][:],
                            in_values=cur[:], imm_value=-1e30)
                        vector.drain()
                        cur = nxt
                vector.drain()
                # base = idx + 441*(idx//7); idx//7 via (idx*9363)>>16
                vector.tensor_copy(out=idx_i[g][:], in_=idx_u[g][:, :_NPATCH])
                vector.drain()
                vector.tensor_scalar(
                    out=br_i[g][:], in0=idx_i[g][:], scalar1=9363,
                    scalar2=None, op0=Op.mult)
                vector.drain()
                vector.tensor_scalar(
                    out=br_i[g][:], in0=br_i[g][:], scalar1=16,
                    scalar2=None, op0=Op.logical_shift_right)
                vector.drain()
                vector.tensor_scalar(
                    out=br_i[g][:], in0=br_i[g][:], scalar1=441,
                    scalar2=None, op0=Op.mult)
                vector.drain()
                vector.tensor_tensor(
                    out=base_bk[g][:], in0=idx_i[g][:], in1=br_i[g][:],
                    op=Op.add)
                vector.drain()
                vector.tensor_copy(out=base_f[g][:], in_=base_bk[g][:])
                vector.drain().then_inc(s_base[g], 1)
                # idx16 slice = f32 add of psD (x4 bcast) + static table,
                # cast to i16 on output
                vector.wait_ge(s_mmD[g], 2)
                vector.tensor_tensor(
                    out=idx16[:, 168 * g:168 * (g + 1)].rearrange(
                        "p (m q) -> p m q", q=4),
                    in0=psD[g][:].rearrange(
                        "p (m one) -> p m one", one=1).to_broadcast(
                        [128, 42, 4]),
                    in1=pk_sb[:, 258 + 168 * g:258 + 168 * (g + 1)].rearrange(
                        "p (m q) -> p m q", q=4),
                    op=Op.add)
                vector.drain().then_inc(s_idx[g], 1)

        @block.tensor
        def _(tensor):
            for g in range(_NGRP):
                tensor.wait_ge(s_red[g], 1)
                if g == 0:
                    tensor.wait_ge(s_cst, 16)
                tensor.matmul(
                    out=psV[g][:],
                    lhsT=pk_sb[:, 0:2],
                    rhs=red[g][:],
                    start=True, stop=True,
                ).then_inc(s_mmV[g], 1)
                # broadcast base across partitions: psD[:, 21b':+21]
                tensor.wait_ge(s_base[g], 1)
                for bl in range(2):
                    tensor.matmul(
                        out=psD[g][:, 21 * bl:21 * (bl + 1)],
                        lhsT=pk_sb[0:2, 2 + 128 * bl:2 + 128 * (bl + 1)],
                        rhs=base_f[g][:],
                        start=True, stop=True,
                    ).then_inc(s_mmD[g], 1)

        @block.gpsimd
        def _(g_):
            from concourse import library_config
            g_.load_library(library_config.mlp)
            # the packed constant rides SWDGE so neither HWDGE ring pays
            # a small-DMA latency at its head
            g_.dma_start(out=pk_sb[:], in_=pk_const[:]).then_inc(s_cst, 16)
            for c, (g, o16, w16, oGT, wGT, n) in enumerate(chunk_geo):
                if c % _NCH_G == 0:
                    g_.wait_ge(s_idx[g], 1)
                g_.dma_gather(
                    out_ap=GT[:, oGT:oGT + wGT, :],
                    in_ap=img_rows,
                    idxs_ap=idx16[:, o16:o16 + w16],
                    num_idxs=n,
                    num_idxs_reg=n,
                    elem_size=_PROW,
                    queue_num=c % 4,
                ).then_inc(s_gq[c % 4], 16)

    nc.finalize()
    _nc_cache = nc
    return nc


def kernel(heatmap, image):
    from concourse.bass_utils import run_bass_kernel_spmd

    heatmap = np.ascontiguousarray(np.asarray(heatmap), dtype=np.float32)
    image = np.ascontiguousarray(np.asarray(image), dtype=np.float32)
    assert heatmap.shape == (_B, 448, 448, 1)
    assert image.shape == (_B, 448, 448, 3)

    nc = build_program()
    in_maps = [
        {
            "heatmap": heatmap[c * _B_LOC:(c + 1) * _B_LOC],
            "image": image[c * _B_LOC:(c + 1) * _B_LOC],
        }
        for c in range(_N_CORES)
    ]
    res = run_bass_kernel_spmd(nc, in_maps, list(range(_N_CORES)))
    outs = [res.results[c]["out"] for c in range(_N_CORES)]
    return np.concatenate(outs, axis=0)


# revision 16
# speedup vs baseline: 1.0844x; 1.0844x over previous
"""Trainium2 Bass kernel for ExtractRelevantPatches (pool -> top-k -> gather).

Full-input contract: kernel(heatmap [64,448,448,1] f32, image [64,448,448,3] f32)
-> [1344, 64, 64, 3] f32.

Sharding: pure data-parallel over batch; 8 batches per NeuronCore, 8 cores.

Per-core algorithm (raw Bass, explicit semaphores), v4 — pipelined over 4
groups of 2 batches, whole top-k/index chain on-chip:

  Per group g (batches 2g, 2g+1; b' = par):
  1. Heatmap -> SBUF [128, 7, 448], partition p = 64*par + r. The two
     partition halves ride DIFFERENT HWDGE rings (par0 on SP/sync, par1 on
     ACT/scalar) so even- and odd-port halves of the SBUF fabric stream
     concurrently (one 64-partition DMA reaches only half the AXI ports).
  2. One DVE reduce_sum over 64-col groups -> red [128, 49].
  3. One matmul with selector F2 [128, 2] (F2[p,i] = 1 iff i == p//64)
     -> PSUM psV [2, 49] = per-batch pooled sums. No DRAM shuffle.
  4. Top-24 via 3x (max + max_index + match_replace); keep first 21.
  5. base[b',k] = idx + 441*(idx//7) (patch-row units), cast to f32.
  6. 2 broadcast matmuls (lhsT = one-hot row selector E2 [2, 128]) spread
     base to all 128 partitions -> PSUM psD [128, 42]; one DVE
     tensor_tensor add (x4 stride-0 broadcast) with f32 static table
     (7*(p%16) + 112*q + 3136*b) -> int16 idx16 slice [128, 168].
  7. dma_gather chunks on 4 SWDGE queues, graduated sizes
     (128,256,512,896,896) per group for a fast ramp (desc-gen runs
     ~8.5ns/idx on one Q7 core per chunk).
  8. One store per gather chunk on the SP ring (free after par0 loads),
     AP out[(c p) e -> p c e]. Per-queue FIFO completion -> 4 counting
     sems cover all 20 chunks.

  All constants ship as ONE packed SWDGE DMA on the gpsimd path so
  neither HWDGE ring pays the small-DMA fixed-latency at its head.
"""

import numpy as np

_N_CORES = 8
_B = 64
_B_LOC = _B // _N_CORES  # 8
_PATCH = 64
_GRID = 7
_NPATCH = 21
_PROW = _PATCH * 3            # 192 elements per patch-row
_OUT_ROWS_LOC = _B_LOC * _NPATCH  # 168
_NGRP = 4                     # batch groups per core
_BG = _B_LOC // _NGRP         # 2 batches per group

# gather chunk sizes per group, in indices (multiples of 128; sum = 2688)
_CHUNKS = [128, 256, 512, 896, 896]
_NCH_G = len(_CHUNKS)
_NCHUNK = _NGRP * _NCH_G      # 20

_nc_cache = None


def build_program():
    """Build the per-core SPMD Bass program (cached)."""
    global _nc_cache
    if _nc_cache is not None:
        return _nc_cache

    import concourse.bass as bass
    import concourse.bacc as bacc
    import concourse.mybir as mybir

    f32 = mybir.dt.float32
    i16 = mybir.dt.int16
    i32 = mybir.dt.int32
    u32 = mybir.dt.uint32
    X = mybir.AxisListType.X
    Op = mybir.AluOpType

    nc = bacc.Bacc(num_swdge_queues=4)

    hm_in = nc.declare_dram_parameter(
        "heatmap", [_B_LOC, 448, 448, 1], f32, isOutput=False)
    img_in = nc.declare_dram_parameter(
        "image", [_B_LOC, 448, 448, 3], f32, isOutput=False)
    out_t = nc.declare_dram_parameter(
        "out", [_OUT_ROWS_LOC, _PATCH, _PATCH, 3], f32, isOutput=True)

    # --- one packed inline constant [128, 930] f32 ------------------------
    # cols 0:2    F2 selector  (F2[p, i] = 1 iff i == p//64)
    # cols 2:258  E2 selectors (E2_bl[p, i] = 1 iff p == bl), partitions 0:2
    # cols 258:930 static gather-index table: position i = R at
    #   [R%16, R//16]; col s: term = 7*(R%16) + 112*(s%4) + 3136*(s//84)
    pk = np.zeros((128, 930), dtype=np.float32)
    pk[:64, 0] = 1.0
    pk[64:, 1] = 1.0
    pk[0, 2:130] = 1.0
    pk[1, 130:258] = 1.0
    s_ar = np.arange(672, dtype=np.int64)
    w_ar = np.arange(16, dtype=np.int64)
    st = (112 * (s_ar[None, :] % 4) + 7 * w_ar[:, None]
          + 3136 * (s_ar[None, :] // 84)).astype(np.float32)
    pk[:, 258:930] = np.tile(st, (8, 1))
    pk_const = nc.inline_tensor(pk, name="pk_const")

    # --- DRAM views -------------------------------------------------------
    img_rows = (img_in[:]
                .rearrange("b r c ch -> (b r c ch)")
                .rearrange("(n e) -> n e", e=_PROW))

    out_pc = (out_t[:]
              .rearrange("r a b c -> (r a b c)")
              .rearrange("(n e) -> n e", e=_PROW)
              .rearrange("(c p) e -> p c e", p=128))

    # heatmap per-group views: [par, 64, 7, 448]
    hm_src = []
    for g in range(_NGRP):
        hm_src.append(
            hm_in[2 * g:2 * (g + 1)]
            .rearrange("par (br r) c one -> par r br (c one)", r=64))

    # per-chunk geometry: (group, idx16 col offset, idx16 col width,
    #                      GT col offset, GT col width, num idxs)
    chunk_geo = []
    for g in range(_NGRP):
        off16 = 168 * g
        offGT = 21 * g
        for n in _CHUNKS:
            chunk_geo.append((g, off16, n // 16, offGT, n // 128, n))
            off16 += n // 16
            offGT += n // 128

    from contextlib import ExitStack

    with ExitStack() as ctx:
        e = ctx.enter_context
        hm = [e(nc.sbuf_tensor(f"hm{g}", [128, 7, 448], f32))
              for g in range(_NGRP)]
        red = [e(nc.sbuf_tensor(f"red{g}", [128, 49], f32))
               for g in range(_NGRP)]
        pk_sb = e(nc.sbuf_tensor("pk_sb", [128, 930], f32))
        V = [e(nc.sbuf_tensor(f"V{g}", [2, 49], f32)) for g in range(_NGRP)]
        vwork = [e(nc.sbuf_tensor(f"vwork{g}", [2, 49], f32))
                 for g in range(_NGRP)]
        m8 = [e(nc.sbuf_tensor(f"m8_{g}", [2, 8], f32)) for g in range(_NGRP)]
        idx_u = [e(nc.sbuf_tensor(f"idx_u{g}", [2, 24], u32))
                 for g in range(_NGRP)]
        idx_i = [e(nc.sbuf_tensor(f"idx_i{g}", [2, _NPATCH], i32))
                 for g in range(_NGRP)]
        br_i = [e(nc.sbuf_tensor(f"br_i{g}", [2, _NPATCH], i32))
                for g in range(_NGRP)]
        base_bk = [e(nc.sbuf_tensor(f"base_bk{g}", [2, _NPATCH], i32))
                   for g in range(_NGRP)]
        base_f = [e(nc.sbuf_tensor(f"base_f{g}", [2, _NPATCH], f32))
                  for g in range(_NGRP)]
        idx16 = e(nc.sbuf_tensor("idx16", [128, 672], i16))
        GT = e(nc.sbuf_tensor("GT", [128, 84, _PROW], f32))
        psV = [e(nc.psum_tensor(f"psV{g}", [2, 49], f32))
               for g in range(_NGRP)]
        psD = [e(nc.psum_tensor(f"psD{g}", [128, 42], f32))
               for g in range(_NGRP)]

        s_ld = [e(nc.semaphore(f"s_ld{g}")) for g in range(_NGRP)]
        s_red = [e(nc.semaphore(f"s_red{g}")) for g in range(_NGRP)]
        s_mmV = [e(nc.semaphore(f"s_mmV{g}")) for g in range(_NGRP)]
        s_base = [e(nc.semaphore(f"s_base{g}")) for g in range(_NGRP)]
        s_mmD = [e(nc.semaphore(f"s_mmD{g}")) for g in range(_NGRP)]
        s_idx = [e(nc.semaphore(f"s_idx{g}")) for g in range(_NGRP)]
        s_cst = e(nc.semaphore("s_cst"))
        s_gq = [e(nc.semaphore(f"s_gq{q}")) for q in range(4)]
        s_st = e(nc.semaphore("s_st"))
        block = e(nc.Block())

        @block.sync
        def _(sync):
            # par0 heatmap halves on the SP ring, then the stores
            for g in range(_NGRP):
                sync.dma_start(
                    out=hm[g][0:64, :, :],
                    in_=hm_src[g][0:1, :, :, :]
                    .rearrange("one r br c -> (one r) br c"),
                ).then_inc(s_ld[g], 16)
            for c, (g, o16, w16, oGT, wGT, n) in enumerate(chunk_geo):
                if c % 2:
                    continue  # odd chunks stored from the ACT ring
                sync.wait_ge(s_gq[c % 4], 16 * (c // 4 + 1))
                sync.dma_start(
                    out=out_pc[:, oGT:oGT + wGT, :],
                    in_=GT[:, oGT:oGT + wGT, :],
                ).then_inc(s_st, 16)
            sync.wait_ge(s_st, 16 * _NCHUNK)

        @block.scalar
        def _(sc):
            # par1 heatmap halves on the ACT ring (nothing else ahead),
            # then the odd-numbered stores
            for g in range(_NGRP):
                sc.dma_start(
                    out=hm[g][64:128, :, :],
                    in_=hm_src[g][1:2, :, :, :]
                    .rearrange("one r br c -> (one r) br c"),
                ).then_inc(s_ld[g], 16)
            for c, (g, o16, w16, oGT, wGT, n) in enumerate(chunk_geo):
                if c % 2 == 0:
                    continue
                sc.wait_ge(s_gq[c % 4], 16 * (c // 4 + 1))
                sc.dma_start(
                    out=out_pc[:, oGT:oGT + wGT, :],
                    in_=GT[:, oGT:oGT + wGT, :],
                ).then_inc(s_st, 16)

        @block.vector
        def _(vector):
            for g in range(_NGRP):
                vector.wait_ge(s_ld[g], 32)
                vector.reduce_sum(
                    out=red[g][:],
                    in_=hm[g][:].rearrange("p br (bc u) -> p (br bc) u", u=64),
                    axis=X,
                )
                vector.drain().then_inc(s_red[g], 1)
                # PSUM -> SBUF pooled sums
                vector.wait_ge(s_mmV[g], 1)
                vector.tensor_copy(out=V[g][:], in_=psV[g][:])
                vector.drain()
                # top-24, keep 21
                cur = V[g]
                for r3 in range(3):
                    vector.max(out=m8[g][:], in_=cur[:])
                    vector.drain()
                    vector.max_index(
                        out=idx_u[g][:, 8 * r3:8 * r3 + 8], in_max=m8[g][:],
                        in_values=cur[:])
                    if r3 < 2:
                        nxt = vwork[g] if r3 == 0 else V[g]
                        vector.match_replace(
                            out=nxt[:], in_to_replace=m8[g][:],
                            in_values=cur[:], imm_value=-1e30)
                        vector.drain()
                        cur = nxt
                vector.drain()
                # base = idx + 441*(idx//7); idx//7 via (idx*9363)>>16
                vector.tensor_copy(out=idx_i[g][:], in_=idx_u[g][:, :_NPATCH])
                vector.drain()
                vector.tensor_scalar(
                    out=br_i[g][:], in0=idx_i[g][:], scalar1=9363,
                    scalar2=None, op0=Op.mult)
                vector.drain()
                vector.tensor_scalar(
                    out=br_i[g][:], in0=br_i[g][:], scalar1=16,
                    scalar2=None, op0=Op.logical_shift_right)
                vector.drain()
                vector.tensor_scalar(
                    out=br_i[g][:], in0=br_i[g][:], scalar1=441,
                    scalar2=None, op0=Op.mult)
                vector.drain()
                vector.tensor_tensor(
                    out=base_bk[g][:], in0=idx_i[g][:], in1=br_i[g][:],
                    op=Op.add)
                vector.drain()
                vector.tensor_copy(out=base_f[g][:], in_=base_bk[g][:])
                vector.drain().then_inc(s_base[g], 1)
                # idx16 slice = f32 add of psD (x4 bcast) + static table,
                # cast to i16 on output
                vector.wait_ge(s_mmD[g], 2)
                vector.tensor_tensor(
                    out=idx16[:, 168 * g:168 * (g + 1)].rearrange(
                        "p (m q) -> p m q", q=4),
                    in0=psD[g][:].rearrange(
                        "p (m one) -> p m one", one=1).to_broadcast(
                        [128, 42, 4]),
                    in1=pk_sb[:, 258 + 168 * g:258 + 168 * (g + 1)].rearrange(
                        "p (m q) -> p m q", q=4),
                    op=Op.add)
                vector.drain().then_inc(s_idx[g], 1)

        @block.tensor
        def _(tensor):
            for g in range(_NGRP):
                tensor.wait_ge(s_red[g], 1)
                if g == 0:
                    tensor.wait_ge(s_cst, 16)
                tensor.matmul(
                    out=psV[g][:],
                    lhsT=pk_sb[:, 0:2],
                    rhs=red[g][:],
                    start=True, stop=True,
                ).then_inc(s_mmV[g], 1)
                # broadcast base across partitions: psD[:, 21b':+21]
                tensor.wait_ge(s_base[g], 1)
                for bl in range(2):
                    tensor.matmul(
                        out=psD[g][:, 21 * bl:21 * (bl + 1)],
                        lhsT=pk_sb[0:2, 2 + 128 * bl:2 + 128 * (bl + 1)],
                        rhs=base_f[g][:],
                        start=True, stop=True,
                    ).then_inc(s_mmD[g], 1)

        @block.gpsimd
        def _(g_):
            from concourse import library_config
            # the packed constant rides SWDGE so neither HWDGE ring pays a
            # small-DMA latency at its head; it must precede load_library,
            # whose ucode overlay blocks the gpsimd queue for ~14us
            g_.dma_start(out=pk_sb[:], in_=pk_const[:]).then_inc(s_cst, 16)
            g_.load_library(library_config.mlp)
            for c, (g, o16, w16, oGT, wGT, n) in enumerate(chunk_geo):
                if c % _NCH_G == 0:
                    g_.wait_ge(s_idx[g], 1)
                g_.dma_gather(
                    out_ap=GT[:, oGT:oGT + wGT, :],
                    in_ap=img_rows,
                    idxs_ap=idx16[:, o16:o16 + w16],
                    num_idxs=n,
                    num_idxs_reg=n,
                    elem_size=_PROW,
                    queue_num=c % 4,
                ).then_inc(s_gq[c % 4], 16)

    nc.finalize()
    _nc_cache = nc
    return nc


def kernel(heatmap, image):
    from concourse.bass_utils import run_bass_kernel_spmd

    heatmap = np.ascontiguousarray(np.asarray(heatmap), dtype=np.float32)
    image = np.ascontiguousarray(np.asarray(image), dtype=np.float32)
    assert heatmap.shape == (_B, 448, 448, 1)
    assert image.shape == (_B, 448, 448, 3)

    nc = build_program()
    in_maps = [
        {
            "heatmap": heatmap[c * _B_LOC:(c + 1) * _B_LOC],
            "image": image[c * _B_LOC:(c + 1) * _B_LOC],
        }
        for c in range(_N_CORES)
    ]
    res = run_bass_kernel_spmd(nc, in_maps, list(range(_N_CORES)))
    outs = [res.results[c]["out"] for c in range(_N_CORES)]
    return np.concatenate(outs, axis=0)


# revision 19
# speedup vs baseline: 1.1044x; 1.0184x over previous
"""Trainium2 Bass kernel for ExtractRelevantPatches (pool -> top-k -> gather).

Full-input contract: kernel(heatmap [64,448,448,1] f32, image [64,448,448,3] f32)
-> [1344, 64, 64, 3] f32.

Sharding: pure data-parallel over batch; 8 batches per NeuronCore, 8 cores.

Per-core algorithm (raw Bass, explicit semaphores), v4 — pipelined over 4
groups of 2 batches, whole top-k/index chain on-chip:

  Per group g (batches 2g, 2g+1; b' = par):
  1. Heatmap -> SBUF [128, 7, 448], partition p = 64*par + r. The two
     partition halves ride DIFFERENT HWDGE rings (par0 on SP/sync, par1 on
     ACT/scalar) so even- and odd-port halves of the SBUF fabric stream
     concurrently (one 64-partition DMA reaches only half the AXI ports).
  2. One DVE reduce_sum over 64-col groups -> red [128, 49].
  3. One matmul with selector F2 [128, 2] (F2[p,i] = 1 iff i == p//64)
     -> PSUM psV [2, 49] = per-batch pooled sums. No DRAM shuffle.
  4. Top-24 via 3x (max + max_index + match_replace); keep first 21.
  5. base[b',k] = idx + 441*(idx//7) (patch-row units), cast to f32.
  6. 2 broadcast matmuls (lhsT = one-hot row selector E2 [2, 128]) spread
     base to all 128 partitions -> PSUM psD [128, 42]; one DVE
     tensor_tensor add (x4 stride-0 broadcast) with f32 static table
     (7*(p%16) + 112*q + 3136*b) -> int16 idx16 slice [128, 168].
  7. dma_gather chunks on 4 SWDGE queues, graduated sizes
     (128,256,512,896,896) per group for a fast ramp (desc-gen runs
     ~8.5ns/idx on one Q7 core per chunk).
  8. One store per gather chunk on the SP ring (free after par0 loads),
     AP out[(c p) e -> p c e]. Per-queue FIFO completion -> 4 counting
     sems cover all 20 chunks.

  All constants ship as ONE packed SWDGE DMA on the gpsimd path so
  neither HWDGE ring pays the small-DMA fixed-latency at its head.
"""

import numpy as np

_N_CORES = 8
_B = 64
_B_LOC = _B // _N_CORES  # 8
_PATCH = 64
_GRID = 7
_NPATCH = 21
_PROW = _PATCH * 3            # 192 elements per patch-row
_OUT_ROWS_LOC = _B_LOC * _NPATCH  # 168
_NGRP = 4                     # batch groups per core
_BG = _B_LOC // _NGRP         # 2 batches per group

# gather chunk sizes per group, in indices (multiples of 128; sum = 2688)
_CHUNKS = [128, 256, 512, 896, 896]
_NCH_G = len(_CHUNKS)
_NCHUNK = _NGRP * _NCH_G      # 20

_nc_cache = None


def build_program():
    """Build the per-core SPMD Bass program (cached)."""
    global _nc_cache
    if _nc_cache is not None:
        return _nc_cache

    import concourse.bass as bass
    import concourse.bacc as bacc
    import concourse.mybir as mybir

    f32 = mybir.dt.float32
    i16 = mybir.dt.int16
    i32 = mybir.dt.int32
    u32 = mybir.dt.uint32
    X = mybir.AxisListType.X
    Op = mybir.AluOpType

    nc = bacc.Bacc(num_swdge_queues=4)

    hm_in = nc.declare_dram_parameter(
        "heatmap", [_B_LOC, 448, 448, 1], f32, isOutput=False)
    img_in = nc.declare_dram_parameter(
        "image", [_B_LOC, 448, 448, 3], f32, isOutput=False)
    out_t = nc.declare_dram_parameter(
        "out", [_OUT_ROWS_LOC, _PATCH, _PATCH, 3], f32, isOutput=True)

    # --- one packed inline constant [128, 930] f32 ------------------------
    # cols 0:2    F2 selector  (F2[p, i] = 1 iff i == p//64)
    # cols 2:258  E2 selectors (E2_bl[p, i] = 1 iff p == bl), partitions 0:2
    # cols 258:930 static gather-index table: position i = R at
    #   [R%16, R//16]; col s: term = 7*(R%16) + 112*(s%4) + 3136*(s//84)
    pk = np.zeros((128, 930), dtype=np.float32)
    pk[:64, 0] = 1.0
    pk[64:, 1] = 1.0
    pk[0, 2:130] = 1.0
    pk[1, 130:258] = 1.0
    s_ar = np.arange(672, dtype=np.int64)
    w_ar = np.arange(16, dtype=np.int64)
    st = (112 * (s_ar[None, :] % 4) + 7 * w_ar[:, None]
          + 3136 * (s_ar[None, :] // 84)).astype(np.float32)
    pk[:, 258:930] = np.tile(st, (8, 1))
    pk_const = nc.inline_tensor(pk, name="pk_const")

    # --- DRAM views -------------------------------------------------------
    img_rows = (img_in[:]
                .rearrange("b r c ch -> (b r c ch)")
                .rearrange("(n e) -> n e", e=_PROW))

    out_pc = (out_t[:]
              .rearrange("r a b c -> (r a b c)")
              .rearrange("(n e) -> n e", e=_PROW)
              .rearrange("(c p) e -> p c e", p=128))

    # heatmap per-group views: [par, 64, 7, 448]
    hm_src = []
    for g in range(_NGRP):
        hm_src.append(
            hm_in[2 * g:2 * (g + 1)]
            .rearrange("par (br r) c one -> par r br (c one)", r=64))

    # per-chunk geometry: (group, idx16 col offset, idx16 col width,
    #                      GT col offset, GT col width, num idxs)
    chunk_geo = []
    for g in range(_NGRP):
        off16 = 168 * g
        offGT = 21 * g
        for n in _CHUNKS:
            chunk_geo.append((g, off16, n // 16, offGT, n // 128, n))
            off16 += n // 16
            offGT += n // 128

    from contextlib import ExitStack

    with ExitStack() as ctx:
        e = ctx.enter_context
        hm = [e(nc.sbuf_tensor(f"hm{g}", [128, 7, 448], f32))
              for g in range(_NGRP)]
        red = [e(nc.sbuf_tensor(f"red{g}", [128, 49], f32))
               for g in range(_NGRP)]
        pk_sb = e(nc.sbuf_tensor("pk_sb", [128, 930], f32))
        V = [e(nc.sbuf_tensor(f"V{g}", [2, 49], f32)) for g in range(_NGRP)]
        vwork = [e(nc.sbuf_tensor(f"vwork{g}", [2, 49], f32))
                 for g in range(_NGRP)]
        m8 = [e(nc.sbuf_tensor(f"m8_{g}", [2, 8], f32)) for g in range(_NGRP)]
        idx_u = [e(nc.sbuf_tensor(f"idx_u{g}", [2, 24], u32))
                 for g in range(_NGRP)]
        br_u = [e(nc.sbuf_tensor(f"br_u{g}", [2, _NPATCH], u32))
                for g in range(_NGRP)]
        br441_f = [e(nc.sbuf_tensor(f"br441_f{g}", [2, _NPATCH], f32))
                   for g in range(_NGRP)]
        idx_f = [e(nc.sbuf_tensor(f"idx_f{g}", [2, _NPATCH], f32))
                 for g in range(_NGRP)]
        idx16 = e(nc.sbuf_tensor("idx16", [128, 672], i16))
        GT = e(nc.sbuf_tensor("GT", [128, 84, _PROW], f32))
        psV = [e(nc.psum_tensor(f"psV{g}", [2, 49], f32))
               for g in range(_NGRP)]
        psD = [e(nc.psum_tensor(f"psD{g}", [128, 42], f32))
               for g in range(_NGRP)]

        s_ld = [e(nc.semaphore(f"s_ld{g}")) for g in range(_NGRP)]
        s_red = [e(nc.semaphore(f"s_red{g}")) for g in range(_NGRP)]
        s_mmV = [e(nc.semaphore(f"s_mmV{g}")) for g in range(_NGRP)]
        s_base = [e(nc.semaphore(f"s_base{g}")) for g in range(_NGRP)]
        s_mmD = [e(nc.semaphore(f"s_mmD{g}")) for g in range(_NGRP)]
        s_idx = [e(nc.semaphore(f"s_idx{g}")) for g in range(_NGRP)]
        s_cst = e(nc.semaphore("s_cst"))
        s_gq = [e(nc.semaphore(f"s_gq{q}")) for q in range(4)]
        s_st = e(nc.semaphore("s_st"))
        block = e(nc.Block())

        @block.sync
        def _(sync):
            # par0 heatmap halves on the SP ring, then the stores
            for g in range(_NGRP):
                sync.dma_start(
                    out=hm[g][0:64, :, :],
                    in_=hm_src[g][0:1, :, :, :]
                    .rearrange("one r br c -> (one r) br c"),
                ).then_inc(s_ld[g], 16)
            for c, (g, o16, w16, oGT, wGT, n) in enumerate(chunk_geo):
                if c % 2:
                    continue  # odd chunks stored from the ACT ring
                sync.wait_ge(s_gq[c % 4], 16 * (c // 4 + 1))
                sync.dma_start(
                    out=out_pc[:, oGT:oGT + wGT, :],
                    in_=GT[:, oGT:oGT + wGT, :],
                ).then_inc(s_st, 16)
            sync.wait_ge(s_st, 16 * _NCHUNK)

        @block.scalar
        def _(sc):
            # par1 heatmap halves on the ACT ring (nothing else ahead),
            # then the odd-numbered stores
            for g in range(_NGRP):
                sc.dma_start(
                    out=hm[g][64:128, :, :],
                    in_=hm_src[g][1:2, :, :, :]
                    .rearrange("one r br c -> (one r) br c"),
                ).then_inc(s_ld[g], 16)
            for c, (g, o16, w16, oGT, wGT, n) in enumerate(chunk_geo):
                if c % 2 == 0:
                    continue
                sc.wait_ge(s_gq[c % 4], 16 * (c // 4 + 1))
                sc.dma_start(
                    out=out_pc[:, oGT:oGT + wGT, :],
                    in_=GT[:, oGT:oGT + wGT, :],
                ).then_inc(s_st, 16)

        @block.vector
        def _(vector):
            for g in range(_NGRP):
                vector.wait_ge(s_ld[g], 32)
                vector.reduce_sum(
                    out=red[g][:],
                    in_=hm[g][:].rearrange("p br (bc u) -> p (br bc) u", u=64),
                    axis=X,
                )
                vector.drain().then_inc(s_red[g], 1)
                # PSUM -> SBUF pooled sums
                vector.wait_ge(s_mmV[g], 1)
                vector.tensor_copy(out=V[g][:], in_=psV[g][:])
                vector.drain()
                # top-24, keep 21
                cur = V[g]
                for r3 in range(3):
                    vector.max(out=m8[g][:], in_=cur[:])
                    vector.drain()
                    vector.max_index(
                        out=idx_u[g][:, 8 * r3:8 * r3 + 8], in_max=m8[g][:],
                        in_values=cur[:])
                    if r3 < 2:
                        nxt = vwork[g] if r3 == 0 else V[g]
                        vector.match_replace(
                            out=nxt[:], in_to_replace=m8[g][:],
                            in_values=cur[:], imm_value=-1e30)
                        vector.drain()
                        cur = nxt
                vector.drain()
                # base = idx + 441*(idx//7); idx//7 via (idx*9363)>>16.
                # No tensor_copy here: an SBUF->SBUF copy is a DVE 2-port
                # op that blocks on the shared SBUF port while a gather
                # desc-gen (GpSimd) holds it, costing ~3.5us per group.
                # The final add rides the PSUM accumulation of the
                # broadcast matmuls instead.
                vector.tensor_scalar(
                    out=br_u[g][:], in0=idx_u[g][:, :_NPATCH], scalar1=9363,
                    scalar2=None, op0=Op.mult)
                vector.drain()
                vector.tensor_scalar(
                    out=br_u[g][:], in0=br_u[g][:], scalar1=16,
                    scalar2=None, op0=Op.logical_shift_right)
                vector.drain()
                vector.tensor_scalar(
                    out=br441_f[g][:], in0=br_u[g][:], scalar1=441,
                    scalar2=None, op0=Op.mult)
                vector.drain()
                vector.tensor_scalar(
                    out=idx_f[g][:], in0=idx_u[g][:, :_NPATCH], scalar1=1,
                    scalar2=None, op0=Op.mult)
                vector.drain().then_inc(s_base[g], 1)
                # idx16 slice = f32 add of psD (x4 bcast) + static table,
                # cast to i16 on output
                vector.wait_ge(s_mmD[g], 2)
                vector.tensor_tensor(
                    out=idx16[:, 168 * g:168 * (g + 1)].rearrange(
                        "p (m q) -> p m q", q=4),
                    in0=psD[g][:].rearrange(
                        "p (m one) -> p m one", one=1).to_broadcast(
                        [128, 42, 4]),
                    in1=pk_sb[:, 258 + 168 * g:258 + 168 * (g + 1)].rearrange(
                        "p (m q) -> p m q", q=4),
                    op=Op.add)
                vector.drain().then_inc(s_idx[g], 1)

        @block.tensor
        def _(tensor):
            for g in range(_NGRP):
                tensor.wait_ge(s_red[g], 1)
                if g == 0:
                    tensor.wait_ge(s_cst, 16)
                tensor.matmul(
                    out=psV[g][:],
                    lhsT=pk_sb[:, 0:2],
                    rhs=red[g][:],
                    start=True, stop=True,
                ).then_inc(s_mmV[g], 1)
                # broadcast base across partitions: psD[:, 21b':+21] =
                # idx + 441*br via two accumulating matmuls per half
                tensor.wait_ge(s_base[g], 1)
                for bl in range(2):
                    tensor.matmul(
                        out=psD[g][:, 21 * bl:21 * (bl + 1)],
                        lhsT=pk_sb[0:2, 2 + 128 * bl:2 + 128 * (bl + 1)],
                        rhs=idx_f[g][:],
                        start=True, stop=False)
                    tensor.matmul(
                        out=psD[g][:, 21 * bl:21 * (bl + 1)],
                        lhsT=pk_sb[0:2, 2 + 128 * bl:2 + 128 * (bl + 1)],
                        rhs=br441_f[g][:],
                        start=False, stop=True,
                    ).then_inc(s_mmD[g], 1)

        @block.gpsimd
        def _(g_):
            from concourse import library_config
            # the packed constant rides SWDGE so neither HWDGE ring pays a
            # small-DMA latency at its head; it must precede load_library,
            # whose ucode overlay blocks the gpsimd queue for ~14us
            g_.dma_start(out=pk_sb[:], in_=pk_const[:]).then_inc(s_cst, 16)
            g_.load_library(library_config.mlp)
            for c, (g, o16, w16, oGT, wGT, n) in enumerate(chunk_geo):
                if c % _NCH_G == 0:
                    g_.wait_ge(s_idx[g], 1)
                g_.dma_gather(
                    out_ap=GT[:, oGT:oGT + wGT, :],
                    in_ap=img_rows,
                    idxs_ap=idx16[:, o16:o16 + w16],
                    num_idxs=n,
                    num_idxs_reg=n,
                    elem_size=_PROW,
                    queue_num=c % 4,
                ).then_inc(s_gq[c % 4], 16)

    nc.finalize()
    _nc_cache = nc
    return nc


def kernel(heatmap, image):
    from concourse.bass_utils import run_bass_kernel_spmd

    heatmap = np.ascontiguousarray(np.asarray(heatmap), dtype=np.float32)
    image = np.ascontiguousarray(np.asarray(image), dtype=np.float32)
    assert heatmap.shape == (_B, 448, 448, 1)
    assert image.shape == (_B, 448, 448, 3)

    nc = build_program()
    in_maps = [
        {
            "heatmap": heatmap[c * _B_LOC:(c + 1) * _B_LOC],
            "image": image[c * _B_LOC:(c + 1) * _B_LOC],
        }
        for c in range(_N_CORES)
    ]
    res = run_bass_kernel_spmd(nc, in_maps, list(range(_N_CORES)))
    outs = [res.results[c]["out"] for c in range(_N_CORES)]
    return np.concatenate(outs, axis=0)


# revision 32
# speedup vs baseline: 1.1166x; 1.0111x over previous
"""Trainium2 Bass kernel for ExtractRelevantPatches (pool -> top-k -> gather).

Full-input contract: kernel(heatmap [64,448,448,1] f32, image [64,448,448,3] f32)
-> [1344, 64, 64, 3] f32.

Sharding: pure data-parallel over batch; 8 batches per NeuronCore, 8 cores.

Per-core algorithm (raw Bass, explicit semaphores), v4 — pipelined over 4
groups of 2 batches, whole top-k/index chain on-chip:

  Per group g (batches 2g, 2g+1; b' = par):
  1. Heatmap -> SBUF [128, 7, 448], partition p = 64*par + r. The two
     partition halves ride DIFFERENT HWDGE rings (par0 on SP/sync, par1 on
     ACT/scalar) so even- and odd-port halves of the SBUF fabric stream
     concurrently (one 64-partition DMA reaches only half the AXI ports).
  2. One DVE reduce_sum over 64-col groups -> red [128, 49].
  3. One matmul with selector F2 [128, 2] (F2[p,i] = 1 iff i == p//64)
     -> PSUM psV [2, 49] = per-batch pooled sums. No DRAM shuffle.
  4. Top-24 via 3x (max + max_index + match_replace); keep first 21.
  5. base[b',k] = idx + 441*(idx//7) (patch-row units), cast to f32.
  6. 2 broadcast matmuls (lhsT = one-hot row selector E2 [2, 128]) spread
     base to all 128 partitions -> PSUM psD [128, 42]; one DVE
     tensor_tensor add (x4 stride-0 broadcast) with f32 static table
     (7*(p%16) + 112*q + 3136*b) -> int16 idx16 slice [128, 168].
  7. dma_gather chunks on 4 SWDGE queues, graduated sizes
     (128,256,512,896,896) per group for a fast ramp (desc-gen runs
     ~8.5ns/idx on one Q7 core per chunk).
  8. One store per gather chunk on the SP ring (free after par0 loads),
     AP out[(c p) e -> p c e]. Per-queue FIFO completion -> 4 counting
     sems cover all 20 chunks.

  All constants ship as ONE packed SWDGE DMA on the gpsimd path so
  neither HWDGE ring pays the small-DMA fixed-latency at its head.
"""

import numpy as np

_N_CORES = 8
_B = 64
_B_LOC = _B // _N_CORES  # 8
_PATCH = 64
_GRID = 7
_NPATCH = 21
_PROW = _PATCH * 3            # 192 elements per patch-row
_OUT_ROWS_LOC = _B_LOC * _NPATCH  # 168
_NGRP = 4                     # batch groups per core
_BG = _B_LOC // _NGRP         # 2 batches per group

# gather chunk sizes per group, in indices (multiples of 128; sum = 2688)
_CHUNKS = [128, 256, 512, 896, 896]
_NCH_G = len(_CHUNKS)
_NCHUNK = _NGRP * _NCH_G      # 20

_nc_cache = None


def build_program():
    """Build the per-core SPMD Bass program (cached)."""
    global _nc_cache
    if _nc_cache is not None:
        return _nc_cache

    import concourse.bass as bass
    import concourse.bacc as bacc
    import concourse.mybir as mybir

    f32 = mybir.dt.float32
    i16 = mybir.dt.int16
    i32 = mybir.dt.int32
    u32 = mybir.dt.uint32
    X = mybir.AxisListType.X
    Op = mybir.AluOpType

    nc = bacc.Bacc(num_swdge_queues=4)

    hm_in = nc.declare_dram_parameter(
        "heatmap", [_B_LOC, 448, 448, 1], f32, isOutput=False)
    img_in = nc.declare_dram_parameter(
        "image", [_B_LOC, 448, 448, 3], f32, isOutput=False)
    out_t = nc.declare_dram_parameter(
        "out", [_OUT_ROWS_LOC, _PATCH, _PATCH, 3], f32, isOutput=True)

    # --- one packed inline constant [128, 1186] f32 -----------------------
    # cols 0:2      F2 selector  (F2[p, i] = 1 iff i == p//64)
    # cols 2:258    A_bl = 65.3125 * E2_bl  (E2_bl[p, i] = 1 iff p == bl)
    # cols 258:514  B_bl = -9.1875 * E2_bl
    #   base = 448*br + bc = 65.3125*pos1 - 9.1875*pos2 exactly in f32,
    #   where pos1 = 7*br + bc (br-major top-k index) and pos2 = 7*bc + br
    #   (bc-major index from a second find_index over a strided view) --
    #   this avoids any integer //7 on the DVE, whose tensor_scalar ops
    #   block on the shared SBUF port while gather desc-gen runs.
    # cols 514:1186 static gather-index table: position i = R at
    #   [R%16, R//16]; col s: term = 7*(R%16) + 112*(s%4) + 3136*(s//84)
    pk = np.zeros((128, 1186), dtype=np.float32)
    pk[:64, 0] = 1.0
    pk[64:, 1] = 1.0
    pk[0, 2:130] = 65.3125
    pk[1, 130:258] = 65.3125
    pk[0, 258:386] = -9.1875
    pk[1, 386:514] = -9.1875
    s_ar = np.arange(672, dtype=np.int64)
    w_ar = np.arange(16, dtype=np.int64)
    st = (112 * (s_ar[None, :] % 4) + 7 * w_ar[:, None]
          + 3136 * (s_ar[None, :] // 84)).astype(np.float32)
    pk[:, 514:1186] = np.tile(st, (8, 1))
    pk_const = nc.inline_tensor(pk, name="pk_const")
    _ST0 = 514  # sttab column offset within pk

    # --- DRAM views -------------------------------------------------------
    img_rows = (img_in[:]
                .rearrange("b r c ch -> (b r c ch)")
                .rearrange("(n e) -> n e", e=_PROW))

    out_pc = (out_t[:]
              .rearrange("r a b c -> (r a b c)")
              .rearrange("(n e) -> n e", e=_PROW)
              .rearrange("(c p) e -> p c e", p=128))

    # heatmap per-group views: [par, 64, 7, 448]
    hm_src = []
    for g in range(_NGRP):
        hm_src.append(
            hm_in[2 * g:2 * (g + 1)]
            .rearrange("par (br r) c one -> par r br (c one)", r=64))

    # per-chunk geometry: (group, idx16 col offset, idx16 col width,
    #                      GT col offset, GT col width, num idxs)
    chunk_geo = []
    for g in range(_NGRP):
        off16 = 168 * g
        offGT = 21 * g
        for n in _CHUNKS:
            chunk_geo.append((g, off16, n // 16, offGT, n // 128, n))
            off16 += n // 16
            offGT += n // 128

    from contextlib import ExitStack

    with ExitStack() as ctx:
        e = ctx.enter_context
        hm = [e(nc.sbuf_tensor(f"hm{g}", [128, 7, 448], f32))
              for g in range(_NGRP)]
        red = [e(nc.sbuf_tensor(f"red{g}", [128, 49], f32))
               for g in range(_NGRP)]
        pk_sb = e(nc.sbuf_tensor("pk_sb", [128, 1186], f32))
        V = [e(nc.sbuf_tensor(f"V{g}", [2, 49], f32)) for g in range(_NGRP)]
        VT = [e(nc.sbuf_tensor(f"VT{g}", [2, 49], f32)) for g in range(_NGRP)]
        vwork = [e(nc.sbuf_tensor(f"vwork{g}", [2, 49], f32))
                 for g in range(_NGRP)]
        vwork2 = [e(nc.sbuf_tensor(f"vwork2_{g}", [2, 49], f32))
                  for g in range(_NGRP)]
        m8 = [e(nc.sbuf_tensor(f"m8_{g}", [2, 8], f32)) for g in range(_NGRP)]
        idx_u = [e(nc.sbuf_tensor(f"idx_u{g}", [2, 24], u32))
                 for g in range(_NGRP)]
        idx_u2 = [e(nc.sbuf_tensor(f"idx_u2_{g}", [2, 24], u32))
                  for g in range(_NGRP)]
        pos1f = [e(nc.sbuf_tensor(f"pos1f{g}", [2, _NPATCH], f32))
                 for g in range(_NGRP)]
        pos2f = [e(nc.sbuf_tensor(f"pos2f{g}", [2, _NPATCH], f32))
                 for g in range(_NGRP)]
        idx16 = e(nc.sbuf_tensor("idx16", [128, 672], i16))
        GT = e(nc.sbuf_tensor("GT", [128, 84, _PROW], f32))
        # single shared PSUM tensors: group g+1's producer matmuls are
        # ordered after group g's consumers via s_red/s_tk program order
        psV_s = e(nc.psum_tensor("psV", [2, 49], f32))
        psVT_s = e(nc.psum_tensor("psVT", [2, 49], f32))
        psD_s = e(nc.psum_tensor("psD", [128, 42], f32))
        psV = [psV_s] * _NGRP
        psVT = [psVT_s] * _NGRP
        psD = [psD_s] * _NGRP

        s_ld = [e(nc.semaphore(f"s_ld{g}")) for g in range(_NGRP)]
        s_red = [e(nc.semaphore(f"s_red{g}")) for g in range(_NGRP)]
        s_tk = [e(nc.semaphore(f"s_tk{g}")) for g in range(_NGRP)]
        s_mmV = [e(nc.semaphore(f"s_mmV{g}")) for g in range(_NGRP)]
        s_base = [e(nc.semaphore(f"s_base{g}")) for g in range(_NGRP)]
        s_mmD = [e(nc.semaphore(f"s_mmD{g}")) for g in range(_NGRP)]
        s_idx = [e(nc.semaphore(f"s_idx{g}")) for g in range(_NGRP)]
        s_cst = e(nc.semaphore("s_cst"))
        s_gq = [e(nc.semaphore(f"s_gq{q}")) for q in range(4)]
        s_st = e(nc.semaphore("s_st"))
        block = e(nc.Block())

        @block.sync
        def _(sync):
            # par0 heatmap halves on the SP ring, then the stores
            for g in range(_NGRP):
                sync.dma_start(
                    out=hm[g][0:64, :, :],
                    in_=hm_src[g][0:1, :, :, :]
                    .rearrange("one r br c -> (one r) br c"),
                ).then_inc(s_ld[g], 16)
            for c, (g, o16, w16, oGT, wGT, n) in enumerate(chunk_geo):
                if c % 2:
                    continue  # odd chunks stored from the ACT ring
                sync.wait_ge(s_gq[c % 4], 16 * (c // 4 + 1))
                sync.dma_start(
                    out=out_pc[:, oGT:oGT + wGT, :],
                    in_=GT[:, oGT:oGT + wGT, :],
                ).then_inc(s_st, 16)
            sync.wait_ge(s_st, 16 * _NCHUNK)

        @block.scalar
        def _(sc):
            # par1 heatmap halves on the ACT ring (nothing else ahead),
            # then the u32->f32 index conversions (ACT has its own SBUF
            # ports, so these never block on gather desc-gen), then the
            # odd-numbered stores
            for g in range(_NGRP):
                sc.dma_start(
                    out=hm[g][64:128, :, :],
                    in_=hm_src[g][1:2, :, :, :]
                    .rearrange("one r br c -> (one r) br c"),
                ).then_inc(s_ld[g], 16)
            Act = mybir.ActivationFunctionType
            for g in range(_NGRP):
                sc.wait_ge(s_tk[g], 1)
                sc.activation(
                    out=pos1f[g][:], in_=idx_u[g][:, :_NPATCH],
                    func=Act.Copy).then_inc(s_base[g], 1)
                sc.activation(
                    out=pos2f[g][:], in_=idx_u2[g][:, :_NPATCH],
                    func=Act.Copy).then_inc(s_base[g], 1)
            for c, (g, o16, w16, oGT, wGT, n) in enumerate(chunk_geo):
                if c % 2 == 0:
                    continue
                sc.wait_ge(s_gq[c % 4], 16 * (c // 4 + 1))
                sc.dma_start(
                    out=out_pc[:, oGT:oGT + wGT, :],
                    in_=GT[:, oGT:oGT + wGT, :],
                ).then_inc(s_st, 16)

        @block.vector
        def _(vector):
            for g in range(_NGRP):
                vector.wait_ge(s_ld[g], 32)
                vector.reduce_sum(
                    out=red[g][:],
                    in_=hm[g][:].rearrange("p br (bc u) -> p (br bc) u", u=64),
                    axis=X,
                )
                vector.drain().then_inc(s_red[g], 1)
                # PSUM -> SBUF pooled sums, br-major and bc-major
                vector.wait_ge(s_mmV[g], 2)
                vector.tensor_copy(out=V[g][:], in_=psV[g][:])
                vector.tensor_copy(out=VT[g][:], in_=psVT[g][:])
                vector.drain()
                # top-24, keep 21; each round also records the bc-major
                # position from the transposed array (find_index searches
                # for m8's values, so it can scan the pristine V/VT), so
                # the patch-row base needs no integer math on the DVE
                cur = V[g]
                for r3 in range(3):
                    vector.max(out=m8[g][:], in_=cur[:])
                    vector.drain()
                    vector.max_index(
                        out=idx_u[g][:, 8 * r3:8 * r3 + 8], in_max=m8[g][:],
                        in_values=V[g][:])
                    vector.max_index(
                        out=idx_u2[g][:, 8 * r3:8 * r3 + 8], in_max=m8[g][:],
                        in_values=VT[g][:])
                    if r3 < 2:
                        nxt = vwork[g] if r3 == 0 else vwork2[g]
                        vector.match_replace(
                            out=nxt[:], in_to_replace=m8[g][:],
                            in_values=cur[:], imm_value=-1e30)
                        vector.drain()
                        cur = nxt
                vector.drain().then_inc(s_tk[g], 1)
                # idx16 slice = f32 add of psD (x4 bcast) + static table,
                # cast to i16 on output
                vector.wait_ge(s_mmD[g], 2)
                vector.tensor_tensor(
                    out=idx16[:, 168 * g:168 * (g + 1)].rearrange(
                        "p (m q) -> p m q", q=4),
                    in0=psD[g][:].rearrange(
                        "p (m one) -> p m one", one=1).to_broadcast(
                        [128, 42, 4]),
                    in1=pk_sb[:, 514 + 168 * g:514 + 168 * (g + 1)].rearrange(
                        "p (m q) -> p m q", q=4),
                    op=Op.add)
                vector.drain().then_inc(s_idx[g], 1)

        @block.tensor
        def _(tensor):
            for g in range(_NGRP):
                tensor.wait_ge(s_red[g], 1)
                if g == 0:
                    tensor.wait_ge(s_cst, 16)
                tensor.matmul(
                    out=psV[g][:],
                    lhsT=pk_sb[:, 0:2],
                    rhs=red[g][:],
                    start=True, stop=True,
                ).then_inc(s_mmV[g], 1)
                tensor.matmul(
                    out=psVT[g][:],
                    lhsT=pk_sb[:, 0:2],
                    rhs=red[g][:].rearrange("p (br bc) -> p bc br", bc=7),
                    start=True, stop=True,
                ).then_inc(s_mmV[g], 1)
                # broadcast base = 65.3125*pos1 - 9.1875*pos2 across
                # partitions via two accumulating matmuls per half with
                # pre-scaled one-hot selectors
                tensor.wait_ge(s_base[g], 2)
                for bl in range(2):
                    tensor.matmul(
                        out=psD[g][:, 21 * bl:21 * (bl + 1)],
                        lhsT=pk_sb[0:2, 2 + 128 * bl:2 + 128 * (bl + 1)],
                        rhs=pos1f[g][:],
                        start=True, stop=False)
                    tensor.matmul(
                        out=psD[g][:, 21 * bl:21 * (bl + 1)],
                        lhsT=pk_sb[0:2, 258 + 128 * bl:258 + 128 * (bl + 1)],
                        rhs=pos2f[g][:],
                        start=False, stop=True,
                    ).then_inc(s_mmD[g], 1)

        @block.gpsimd
        def _(g_):
            from concourse import library_config
            # the packed constant rides SWDGE so neither HWDGE ring pays a
            # small-DMA latency at its head; it must precede load_library,
            # whose ucode overlay blocks the gpsimd queue for ~14us
            g_.dma_start(out=pk_sb[:], in_=pk_const[:]).then_inc(s_cst, 16)
            g_.load_library(library_config.mlp)
            for c, (g, o16, w16, oGT, wGT, n) in enumerate(chunk_geo):
                if c % _NCH_G == 0:
                    g_.wait_ge(s_idx[g], 1)
                g_.dma_gather(
                    out_ap=GT[:, oGT:oGT + wGT, :],
                    in_ap=img_rows,
                    idxs_ap=idx16[:, o16:o16 + w16],
                    num_idxs=n,
                    num_idxs_reg=n,
                    elem_size=_PROW,
                    queue_num=c % 4,
                ).then_inc(s_gq[c % 4], 16)

    nc.finalize()
    _nc_cache = nc
    return nc


def kernel(heatmap, image):
    from concourse.bass_utils import run_bass_kernel_spmd

    heatmap = np.ascontiguousarray(np.asarray(heatmap), dtype=np.float32)
    image = np.ascontiguousarray(np.asarray(image), dtype=np.float32)
    assert heatmap.shape == (_B, 448, 448, 1)
    assert image.shape == (_B, 448, 448, 3)

    nc = build_program()
    in_maps = [
        {
            "heatmap": heatmap[c * _B_LOC:(c + 1) * _B_LOC],
            "image": image[c * _B_LOC:(c + 1) * _B_LOC],
        }
        for c in range(_N_CORES)
    ]
    res = run_bass_kernel_spmd(nc, in_maps, list(range(_N_CORES)))
    outs = [res.results[c]["out"] for c in range(_N_CORES)]
    return np.concatenate(outs, axis=0)
